# revision 1
# baseline (speedup 1.0000x reference)
# BinsCombinerLayer Trainium2 kernel.
#
#   out[b] = (1/NUM_BINS) * sum_{n,s} inputs[b,n,s] * centroids[n,s]
#
# Pure data parallel over 8 NeuronCores: each core takes B/8 = 4096 examples.
# Per core, the slice is viewed as [128 partitions, 32 examples-per-partition,
# 2048 elems] with example b = 32*p + t, so every DMA reads one contiguous
# multi-example run per partition and the final [128, 32] result tile maps
# contiguously back to DRAM.  The dot product against the centroid table
# (pre-scaled by 1/NUM_BINS on host, broadcast to all 128 partitions on-chip
# via a K=1 ones matmul) is one fused DVE scalar_tensor_tensor per example
# row: out = (x * 1.0) * c elementwise (written to a stride-0 dummy),
# accum_out = free-axis sum.
import numpy as np

import concourse.bacc as bacc
import concourse.mybir as mybir
import concourse.tile as tile
from concourse.bass_utils import run_bass_kernel_spmd

N_CORES = 8
B, NUM_BINS, BIN_SIZE = 32768, 16, 128
D = NUM_BINS * BIN_SIZE      # 2048 contiguous f32 per example
P = 128                      # SBUF partitions
BC = B // N_CORES            # 4096 examples per core
T = BC // P                  # 32 examples per partition
F32 = mybir.dt.float32

_CACHED = None


def _build_program(repeat=1, tile_g=2, bufs=6, split_tail=True):
    nc = bacc.Bacc("TRN2", target_bir_lowering=False, debug=False)
    x = nc.dram_tensor("x", [P, T * D], F32, kind="ExternalInput").ap()
    cb = nc.dram_tensor("cb", [1, D], F32, kind="ExternalInput").ap()
    out = nc.dram_tensor("out", [P, T], F32, kind="ExternalOutput").ap()

    # per-pass schedule: tile_g examples per DMA, optionally splitting the
    # final two groups into single-example DMAs to shorten the DVE trail
    # after the last DMA completes.
    groups = [tile_g] * (T // tile_g)
    if split_tail and tile_g > 1 and len(groups) >= 1:
        groups = groups[:-1] + [1] * tile_g

    with tile.TileContext(nc) as tc:
        with (
            tc.tile_pool(name="xin", bufs=bufs) as xpool,
            tc.tile_pool(name="misc", bufs=1) as misc,
            tc.tile_pool(name="ps", bufs=1, space="PSUM") as pspool,
        ):
            # broadcast the [1, D] centroid row to all 128 partitions:
            # ones[1, P].T @ cb[1, D] -> psum[P, D], then copy to SBUF.
            cbs = misc.tile([1, D], F32)
            nc.gpsimd.dma_start(out=cbs[:], in_=cb[:])
            ones = misc.tile([1, P], F32)
            nc.gpsimd.memset(ones[:], 1.0)
            cbp = pspool.tile([P, D], F32)
            for k in range(D // 512):
                nc.tensor.matmul(
                    cbp[:, k * 512 : (k + 1) * 512],
                    ones[:],
                    cbs[:, k * 512 : (k + 1) * 512],
                    start=True,
                    stop=True,
                )
            cbt = misc.tile([P, D], F32)
            nc.scalar.copy(cbt[:], cbp[:])

            collect = misc.tile([P, T], F32)
            # STT's discarded elementwise result goes to a stride-0 PSUM
            # dummy: PSUM has its own write ports, so the ~2B/elem/cycle of
            # dummy writes stop contending with the DMA's SBUF fill.
            dummy = pspool.tile([P, 1], F32)

            for _ in range(repeat):
                t = 0
                for g_sz in groups:
                    xt = xpool.tile([P, tile_g * D], F32, tag="xt")
                    nc.sync.dma_start(
                        out=xt[:, : g_sz * D],
                        in_=x[:, t * D : (t + g_sz) * D],
                    )
                    for g in range(g_sz):
                        nc.vector.scalar_tensor_tensor(
                            out=dummy.broadcast_to((P, D)),
                            in0=xt[:, g * D : (g + 1) * D],
                            scalar=1.0,
                            in1=cbt[:],
                            op0=mybir.AluOpType.mult,
                            op1=mybir.AluOpType.mult,
                            accum_out=collect[:, t + g : t + g + 1],
                        )
                    t += g_sz

            nc.sync.dma_start(out=out[:], in_=collect[:])

    nc.compile()
    return nc


def _get_program():
    global _CACHED
    if _CACHED is None:
        _CACHED = _build_program()
    return _CACHED


def run(inputs, centroids, **spmd_kwargs):
    """Run the kernel; returns (full_output, BassKernelResults)."""
    nc = _get_program()
    scaled = np.asarray(centroids, dtype=np.float32).reshape(1, D) / NUM_BINS
    cbv = np.ascontiguousarray(scaled)
    x = np.ascontiguousarray(inputs, dtype=np.float32).reshape(
        N_CORES, P, T * D
    )
    in_maps = [{"x": x[i], "cb": cbv} for i in range(N_CORES)]
    try:
        res = run_bass_kernel_spmd(
            nc, in_maps, list(range(N_CORES)), **spmd_kwargs
        )
    except Exception:
        # transient NRT_EXEC_UNIT_UNRECOVERABLE wedges recover on retry
        res = run_bass_kernel_spmd(
            nc, in_maps, list(range(N_CORES)), **spmd_kwargs
        )
    full = np.concatenate([r["out"].reshape(BC) for r in res.results])
    return full.astype(np.float32, copy=False), res


def kernel(inputs, centroids):
    full, _ = run(inputs, centroids)
    return full

